# revision 5
# baseline (speedup 1.0000x reference)
"""Dcls2d depthwise conv (learnable-spacing dilated conv) for Trainium2.

Math: P1/P2 are (1,3,3) -> tap positions shared across all 384 channels.
The 21x21 constructed kernel is a bilinear scatter of the 3x3 weight grid,
so the conv is a sum of <=36 integer-shifted copies of the input, each
scaled by a per-channel coefficient:

    out[n,c,y,x] = bias[c] + sum_j coef[c,j] * in[n,c,y+dy_j,x+dx_j]

Sharding: data-parallel over batch, 32 imgs -> 4 per core on 8 cores.
Shift positions are computed on host from P1/P2 (tiny) and baked into the
compiled kernel's access patterns; per-channel coefficients are a runtime
input tensor.
"""

import os
import time
from contextlib import ExitStack

import numpy as np

import concourse.bass as bass
import concourse.tile as tile
from concourse import bacc, mybir
from concourse.bass_utils import run_bass_kernel_spmd

F32 = mybir.dt.float32
ALU = mybir.AluOpType

N, C, H, W = 32, 384, 56, 56
NCORES = 8
NPER = N // NCORES  # 4 images per core
K0 = K1 = 3
D0 = D1 = 7
L0 = L1 = 21  # constructed kernel size
PAD = 10
NBLK = C // 128  # 3 channel blocks

# taps assigned to the TensorEngine (diagonal matmuls, float32r) vs the
# VectorEngine (scalar_tensor_tensor fp32); tuned on HW profile.
PE_FRACTION = 1.0


def _host_taps(weight, P1, P2):
    """Bilinear scatter on host -> list of ((dy, dx), coef[384]) taps."""
    w = np.asarray(weight, np.float64).reshape(C, K0 * K1)  # Cg == 1
    p1 = np.clip(np.asarray(P1, np.float64).reshape(-1) + L0 // 2, 0.0, L0 - 1.0)
    p2 = np.clip(np.asarray(P2, np.float64).reshape(-1) + L1 // 2, 0.0, L1 - 1.0)
    f1, f2 = np.floor(p1), np.floor(p2)
    r1, r2 = p1 - f1, p2 - f2
    i1, i2 = f1.astype(int), f2.astype(int)
    i1p = np.minimum(i1 + 1, L0 - 1)
    i2p = np.minimum(i2 + 1, L1 - 1)

    acc = {}  # (a, b) -> coef vector (float64)
    for t in range(K0 * K1):
        for a, b, cf in (
            (i1[t], i2[t], (1 - r1[t]) * (1 - r2[t])),
            (i1p[t], i2[t], r1[t] * (1 - r2[t])),
            (i1[t], i2p[t], (1 - r1[t]) * r2[t]),
            (i1p[t], i2p[t], r1[t] * r2[t]),
        ):
            key = (int(a), int(b))
            v = acc.setdefault(key, np.zeros(C, np.float64))
            v += w[:, t] * cf

    taps = [((a - PAD, b - PAD), v) for (a, b), v in sorted(acc.items())]
    return taps


def _build(taps, bias_np=None):
    """Build + compile the per-core Bass program. taps: list of (dy, dx)."""
    ntaps = len(taps)
    nc = bacc.Bacc("TRN2", target_bir_lowering=False, debug=False,
                   num_devices=NCORES)
    x = nc.dram_tensor("x", (NPER, C, H, W), F32, kind="ExternalInput").ap()
    coefs = nc.dram_tensor("coefs", (C, ntaps), F32, kind="ExternalInput").ap()
    biasb = nc.dram_tensor("biasb", (C, 1), F32, kind="ExternalInput").ap()
    out = nc.dram_tensor("out", (NPER, C, H, W), F32, kind="ExternalOutput").ap()

    with tile.TileContext(nc) as tc, ExitStack() as ctx:
        cpool = ctx.enter_context(tc.tile_pool(name="const", bufs=NBLK))
        xpool = ctx.enter_context(tc.tile_pool(name="xin", bufs=3))
        apool = ctx.enter_context(tc.tile_pool(name="acc", bufs=3))

        ct, bt = [], []
        for b in range(NBLK):
            c_t = cpool.tile([128, ntaps], F32, tag="coef")
            nc.sync.dma_start(c_t[:], coefs[128 * b:128 * (b + 1), :])
            ct.append(c_t)
            b_t = cpool.tile([128, 1], F32, tag="bias")
            nc.sync.dma_start(b_t[:], biasb[128 * b:128 * (b + 1), :])
            bt.append(b_t)

        for i in range(NPER):
            for b in range(NBLK):
                xt = xpool.tile([128, H * W], F32)
                nc.sync.dma_start(
                    xt[:],
                    x[i, 128 * b:128 * (b + 1)].rearrange("c h w -> c (h w)"))
                acc = apool.tile([128, H * W], F32)
                # acc = x*0 + bias  (broadcast per-partition bias)
                nc.vector.tensor_scalar(acc[:], xt[:], 0.0, bt[b][:, 0:1],
                                        ALU.mult, ALU.add)
                x3 = xt[:].rearrange("c (h w) -> c h w", w=W)
                a3 = acc[:].rearrange("c (h w) -> c h w", w=W)
                for t, (dy, dx) in enumerate(taps):
                    y0, y1 = max(0, -dy), min(H, H - dy)
                    x0, x1 = max(0, -dx), min(W, W - dx)
                    av = a3[:, y0:y1, x0:x1]
                    xv = x3[:, y0 + dy:y1 + dy, x0 + dx:x1 + dx]
                    # acc += coef[c,t] * x_shifted   (one fused DVE op)
                    nc.vector.scalar_tensor_tensor(
                        av, xv, ct[b][:, t:t + 1], av, ALU.mult, ALU.add)
                nc.sync.dma_start(
                    out[i, 128 * b:128 * (b + 1)].rearrange("c h w -> c (h w)"),
                    acc[:])

    nc.compile()
    return nc


def _dispatch(nc, in_maps, time_iters=0):
    """Run the compiled Bass module on NCORES cores via PJRT (axon path),
    mirroring bass2jax.run_bass_via_pjrt but with optional repeat-timing on
    device-resident inputs. Returns (results_list, per_call_seconds)."""
    import jax
    from jax.sharding import Mesh, PartitionSpec
    from jax.experimental.shard_map import shard_map
    from concourse import bass2jax, mybir as _mybir
    from concourse.bass2jax import _bass_exec_p, install_neuronx_cc_hook

    install_neuronx_cc_hook()
    n_cores = len(in_maps)

    partition_name = (nc.partition_id_tensor.name
                      if nc.partition_id_tensor else None)
    in_names, out_names, out_avals, zero_outs = [], [], [], []
    for alloc in nc.m.functions[0].allocations:
        if not isinstance(alloc, _mybir.MemoryLocationSet):
            continue
        name = alloc.memorylocations[0].name
        if alloc.kind == "ExternalInput":
            if name != partition_name:
                in_names.append(name)
        elif alloc.kind == "ExternalOutput":
            shape = tuple(alloc.tensor_shape)
            dtype = _mybir.dt.np(alloc.dtype)
            out_names.append(name)
            out_avals.append(jax.core.ShapedArray(shape, dtype))
            zero_outs.append(np.zeros(shape, dtype))
    n_params = len(in_names)
    all_names = in_names + out_names
    if partition_name is not None:
        all_names = all_names + [partition_name]

    def _body(*args):
        operands = list(args)
        if partition_name is not None:
            operands.append(bass2jax.partition_id_tensor())
        outs = _bass_exec_p.bind(
            *operands,
            out_avals=tuple(out_avals),
            in_names=tuple(all_names),
            out_names=tuple(out_names),
            lowering_input_output_aliases=(),
            sim_require_finite=True,
            sim_require_nnan=True,
            nc=nc,
        )
        return tuple(outs)

    devices = jax.devices()[:n_cores]
    mesh = Mesh(np.asarray(devices), ("core",))
    n_args = n_params + len(out_names)
    sharded = jax.jit(
        shard_map(_body, mesh=mesh,
                  in_specs=(PartitionSpec("core"),) * n_args,
                  out_specs=(PartitionSpec("core"),) * len(out_names),
                  check_rep=False),
        keep_unused=True,
    )
    concat_in = [
        np.concatenate([np.asarray(m[name]) for m in in_maps], axis=0)
        for name in in_names
    ]
    concat_zero = [
        np.zeros((n_cores * z.shape[0], *z.shape[1:]), z.dtype) for z in zero_outs
    ]
    sharding = jax.sharding.NamedSharding(mesh, PartitionSpec("core"))
    dev_args = [jax.device_put(a, sharding) for a in concat_in + concat_zero]

    out_arrs = jax.block_until_ready(sharded(*dev_args))
    times = []
    for _ in range(time_iters):
        t0 = time.perf_counter()
        jax.block_until_ready(sharded(*dev_args))
        times.append(time.perf_counter() - t0)

    results = [
        {name: np.asarray(out_arrs[i]).reshape(n_cores, *out_avals[i].shape)[c]
         for i, name in enumerate(out_names)}
        for c in range(n_cores)
    ]
    return results, times


def _null_nc():
    """Tiny kernel through the same path — measures per-call dispatch floor."""
    nc = bacc.Bacc("TRN2", target_bir_lowering=False, debug=False,
                   num_devices=NCORES)
    x = nc.dram_tensor("x", (128, 128), F32, kind="ExternalInput").ap()
    out = nc.dram_tensor("out", (128, 128), F32, kind="ExternalOutput").ap()
    with tile.TileContext(nc) as tc, ExitStack() as ctx:
        pool = ctx.enter_context(tc.tile_pool(name="p", bufs=1))
        t = pool.tile([128, 128], F32)
        nc.sync.dma_start(t[:], x[:])
        nc.sync.dma_start(out[:], t[:])
    nc.compile()
    return nc


def _prep(input, weight, P1, P2, bias):
    input = np.ascontiguousarray(np.asarray(input, np.float32))
    taps = _host_taps(weight, P1, P2)
    positions = [p for p, _ in taps]
    coefs = np.stack([v for _, v in taps], axis=1).astype(np.float32)  # (C, ntaps)
    bias_col = np.asarray(bias, np.float32).reshape(C, 1)
    in_maps = [
        {"x": input[i * NPER:(i + 1) * NPER], "coefs": coefs, "biasb": bias_col}
        for i in range(NCORES)
    ]
    return positions, in_maps


def _run(input, weight, P1, P2, bias, time_iters=0):
    positions, in_maps = _prep(input, weight, P1, P2, bias)
    nc = _build(positions)
    results, times = _dispatch(nc, in_maps, time_iters=time_iters)
    full = np.concatenate([r["out"] for r in results], axis=0)
    return full, times


def kernel(input, weight, P1, P2, bias):
    full, _ = _run(input, weight, P1, P2, bias)
    return full


# revision 12
# speedup vs baseline: 1.0140x; 1.0140x over previous
"""Dcls2d depthwise conv (learnable-spacing dilated conv) for Trainium2.

Math: P1/P2 are (1,3,3) -> tap positions shared across all 384 channels.
The 21x21 constructed kernel is a bilinear scatter of the 3x3 weight grid,
so the conv is a sum of <=36 integer-shifted copies of the input, each
scaled by a per-channel coefficient:

    out[n,c,y,x] = bias[c] + sum_j coef[c,j] * in[n,c,y+dy_j,x+dx_j]

Sharding: data-parallel over batch, 32 imgs -> 4 per core on 8 cores.
Shift positions are computed on host from P1/P2 (tiny) and baked into the
compiled kernel's access patterns; per-channel coefficients are a runtime
input tensor.
"""

import os
import time
from contextlib import ExitStack

import numpy as np

import concourse.bass as bass
import concourse.tile as tile
from concourse import bacc, mybir
from concourse.bass_utils import run_bass_kernel_spmd

F32 = mybir.dt.float32
F32R = mybir.dt.float32r
ALU = mybir.AluOpType

N, C, H, W = 32, 384, 56, 56
NCORES = 8
NPER = N // NCORES  # 4 images per core
K0 = K1 = 3
D0 = D1 = 7
L0 = L1 = 21  # constructed kernel size
PAD = 10
NBLK = C // 128  # 3 channel blocks
HP, WP = H + 2 * PAD, W + 2 * PAD  # 76x76 padded tile
RPC = 8                      # output rows per PSUM chunk
NCHUNK = H // RPC            # 7 chunks of 8*56=448 columns
CHW = RPC * W                # 448

# how many taps run as exact-fp32 DVE FMAs (the rest go to the
# TensorEngine as float32r diagonal matmuls); tuned on the cost model.
NDVE = 10


def _host_taps(weight, P1, P2):
    """Bilinear scatter on host -> list of ((dy, dx), coef[384]) taps."""
    w = np.asarray(weight, np.float64).reshape(C, K0 * K1)  # Cg == 1
    p1 = np.clip(np.asarray(P1, np.float64).reshape(-1) + L0 // 2, 0.0, L0 - 1.0)
    p2 = np.clip(np.asarray(P2, np.float64).reshape(-1) + L1 // 2, 0.0, L1 - 1.0)
    f1, f2 = np.floor(p1), np.floor(p2)
    r1, r2 = p1 - f1, p2 - f2
    i1, i2 = f1.astype(int), f2.astype(int)
    i1p = np.minimum(i1 + 1, L0 - 1)
    i2p = np.minimum(i2 + 1, L1 - 1)

    acc = {}  # (a, b) -> coef vector (float64)
    for t in range(K0 * K1):
        for a, b, cf in (
            (i1[t], i2[t], (1 - r1[t]) * (1 - r2[t])),
            (i1p[t], i2[t], r1[t] * (1 - r2[t])),
            (i1[t], i2p[t], (1 - r1[t]) * r2[t]),
            (i1p[t], i2p[t], r1[t] * r2[t]),
        ):
            key = (int(a), int(b))
            v = acc.setdefault(key, np.zeros(C, np.float64))
            v += w[:, t] * cf

    taps = [((a - PAD, b - PAD), v) for (a, b), v in sorted(acc.items())]
    return taps


def _build_hybrid(dve_taps, pe_taps, reps=1):
    """Hybrid TensorE+VectorE per-core program.

    dve_taps: list of (dy, dx) done as exact-fp32 scalar_tensor_tensor on DVE
    pe_taps:  list of (dy, dx) done as float32r diagonal matmuls on TensorE,
              accumulated in PSUM (7 banks = 7 row-chunks per image-block)
    Inputs: x (NPER,C,H,W) f32; coefs (C, n_dve) f32; diags (NBLK*n_pe,128,128)
    f32; biasb (C,1) f32; zeros (128, 768) f32.
    """
    n_dve, n_pe = len(dve_taps), len(pe_taps)
    nc = bacc.Bacc("TRN2", target_bir_lowering=False, debug=False,
                   num_devices=NCORES)
    x = nc.dram_tensor("x", (NPER, C, H, W), F32, kind="ExternalInput").ap()
    coefs = nc.dram_tensor("coefs", (C, max(n_dve, 1)), F32,
                           kind="ExternalInput").ap()
    diags = nc.dram_tensor("diags", (NBLK * n_pe, 128, 128), F32,
                           kind="ExternalInput").ap()
    biasb = nc.dram_tensor("biasb", (C, 1), F32, kind="ExternalInput").ap()
    zeros = nc.dram_tensor("zeros", (128, WP * PAD), F32,
                           kind="ExternalInput").ap()
    out = nc.dram_tensor("out", (NPER, C, H, W), F32, kind="ExternalOutput").ap()

    with tile.TileContext(nc) as tc, ExitStack() as ctx:
        cpool = ctx.enter_context(tc.tile_pool(name="const", bufs=NBLK))
        dpool = ctx.enter_context(tc.tile_pool(name="diag", bufs=NBLK * n_pe))
        stpool = ctx.enter_context(tc.tile_pool(name="stage", bufs=2))
        xpool = ctx.enter_context(tc.tile_pool(name="xin", bufs=2))
        ppool = ctx.enter_context(tc.tile_pool(name="pad", bufs=2))
        apool = ctx.enter_context(tc.tile_pool(name="acc", bufs=2))
        pspool = ctx.enter_context(tc.tile_pool(name="psum", bufs=8,
                                                space="PSUM"))

        zb = zeros.bitcast(F32R)

        ct, bt = [], []
        for b in range(NBLK):
            c_t = cpool.tile([128, max(n_dve, 1)], F32, tag="coef")
            nc.sync.dma_start(c_t[:], coefs[128 * b:128 * (b + 1), :])
            ct.append(c_t)
            b_t = cpool.tile([128, 1], F32, tag="bias")
            nc.sync.dma_start(b_t[:], biasb[128 * b:128 * (b + 1), :])
            bt.append(b_t)

        # stage + round the per-tap diagonal weight matrices to float32r
        dg = []
        for k in range(NBLK * n_pe):
            st = stpool.tile([128, 128], F32, tag="dstage")
            nc.sync.dma_start(st[:], diags[k])
            d_t = dpool.tile([128, 128], F32R, tag="diag")
            nc.vector.tensor_copy(d_t[:], st[:])
            dg.append(d_t)

        rep_ctx = tc.For_i(0, reps, 1) if reps > 1 else None
        if rep_ctx is not None:
            ctx.enter_context(rep_ctx)
        for i in range(NPER):
            for b in range(NBLK):
                xt = xpool.tile([128, H * W], F32)
                nc.sync.dma_start(
                    xt[:],
                    x[i, 128 * b:128 * (b + 1)].rearrange("c h w -> c (h w)"))

                # padded float32r copy of the image block
                xp = ppool.tile([128, HP * WP], F32R, tag="xpad")
                xp3 = xp[:].rearrange("c (h w) -> c h w", w=WP)
                # zero borders (top, bottom, left, right) via DMA
                nc.sync.dma_start(xp3[:, 0:PAD, :].rearrange("c h w -> c (h w)"),
                                  zb[:, :])
                nc.sync.dma_start(
                    xp3[:, H + PAD:HP, :].rearrange("c h w -> c (h w)"),
                    zb[:, :])
                nc.sync.dma_start(xp3[:, PAD:PAD + H, 0:PAD],
                                  zb.rearrange("c (h w) -> c h w", w=PAD)[:, :H, :])
                nc.sync.dma_start(xp3[:, PAD:PAD + H, W + PAD:WP],
                                  zb.rearrange("c (h w) -> c h w", w=PAD)[:, :H, :])
                # round interior fp32 -> f32r (DVE copy)
                nc.vector.tensor_copy(
                    xp3[:, PAD:PAD + H, PAD:PAD + W],
                    xt[:].rearrange("c (h w) -> c h w", w=W))

                xpf = xp[:].bitcast(F32).rearrange("c (h w) -> c h w", w=WP)

                # --- TensorE: per-tap diagonal matmuls, tap-outer so each
                # diag loads once per image-block; 7 PSUM banks accumulate ---
                pst = [pspool.tile([128, CHW], F32, tag="ps", name=f"ps{cix}")
                       for cix in range(NCHUNK)]
                for k, (dy, dx) in enumerate(pe_taps):
                    d_t = dg[b * n_pe + k]
                    for cix in range(NCHUNK):
                        rhs = xp3[:, RPC * cix + PAD + dy:RPC * (cix + 1) + PAD + dy,
                                  PAD + dx:PAD + dx + W]
                        nc.tensor.matmul(pst[cix][:], d_t[:], rhs,
                                         start=(k == 0), stop=(k == n_pe - 1))

                # --- VectorE: exact fp32 taps into SBUF accumulator ---
                acc = apool.tile([128, H * W], F32)
                a3 = acc[:].rearrange("c (h w) -> c h w", w=W)
                for t, (dy, dx) in enumerate(dve_taps):
                    if t == 0:
                        # full rect: acc = coef * x_shift  (borders read zeros)
                        nc.vector.tensor_scalar(
                            a3[:, :, :],
                            xpf[:, PAD + dy:PAD + dy + H, PAD + dx:PAD + dx + W],
                            ct[b][:, 0:1], None, ALU.mult)
                        continue
                    y0, y1 = max(0, -dy), min(H, H - dy)
                    x0, x1 = max(0, -dx), min(W, W - dx)
                    av = a3[:, y0:y1, x0:x1]
                    xv = xpf[:, PAD + y0 + dy:PAD + y1 + dy,
                             PAD + x0 + dx:PAD + x1 + dx]
                    nc.vector.scalar_tensor_tensor(
                        av, xv, ct[b][:, t:t + 1], av, ALU.mult, ALU.add)

                # --- merge PSUM + acc + bias, then store ---
                for cix in range(NCHUNK):
                    ac = acc[:, CHW * cix:CHW * (cix + 1)]
                    nc.vector.scalar_tensor_tensor(
                        ac, pst[cix][:], bt[b][:, 0:1], ac, ALU.add, ALU.add)
                nc.sync.dma_start(
                    out[i, 128 * b:128 * (b + 1)].rearrange("c h w -> c (h w)"),
                    acc[:])

    nc.compile()
    return nc


def _build(taps, bias_np=None):
    """Build + compile the per-core Bass program. taps: list of (dy, dx)."""
    ntaps = len(taps)
    nc = bacc.Bacc("TRN2", target_bir_lowering=False, debug=False,
                   num_devices=NCORES)
    x = nc.dram_tensor("x", (NPER, C, H, W), F32, kind="ExternalInput").ap()
    coefs = nc.dram_tensor("coefs", (C, ntaps), F32, kind="ExternalInput").ap()
    biasb = nc.dram_tensor("biasb", (C, 1), F32, kind="ExternalInput").ap()
    out = nc.dram_tensor("out", (NPER, C, H, W), F32, kind="ExternalOutput").ap()

    with tile.TileContext(nc) as tc, ExitStack() as ctx:
        cpool = ctx.enter_context(tc.tile_pool(name="const", bufs=NBLK))
        xpool = ctx.enter_context(tc.tile_pool(name="xin", bufs=3))
        apool = ctx.enter_context(tc.tile_pool(name="acc", bufs=3))

        ct, bt = [], []
        for b in range(NBLK):
            c_t = cpool.tile([128, ntaps], F32, tag="coef")
            nc.sync.dma_start(c_t[:], coefs[128 * b:128 * (b + 1), :])
            ct.append(c_t)
            b_t = cpool.tile([128, 1], F32, tag="bias")
            nc.sync.dma_start(b_t[:], biasb[128 * b:128 * (b + 1), :])
            bt.append(b_t)

        for i in range(NPER):
            for b in range(NBLK):
                xt = xpool.tile([128, H * W], F32)
                nc.sync.dma_start(
                    xt[:],
                    x[i, 128 * b:128 * (b + 1)].rearrange("c h w -> c (h w)"))
                acc = apool.tile([128, H * W], F32)
                # acc = x*0 + bias  (broadcast per-partition bias)
                nc.vector.tensor_scalar(acc[:], xt[:], 0.0, bt[b][:, 0:1],
                                        ALU.mult, ALU.add)
                x3 = xt[:].rearrange("c (h w) -> c h w", w=W)
                a3 = acc[:].rearrange("c (h w) -> c h w", w=W)
                for t, (dy, dx) in enumerate(taps):
                    y0, y1 = max(0, -dy), min(H, H - dy)
                    x0, x1 = max(0, -dx), min(W, W - dx)
                    av = a3[:, y0:y1, x0:x1]
                    xv = x3[:, y0 + dy:y1 + dy, x0 + dx:x1 + dx]
                    # acc += coef[c,t] * x_shifted   (one fused DVE op)
                    nc.vector.scalar_tensor_tensor(
                        av, xv, ct[b][:, t:t + 1], av, ALU.mult, ALU.add)
                nc.sync.dma_start(
                    out[i, 128 * b:128 * (b + 1)].rearrange("c h w -> c (h w)"),
                    acc[:])

    nc.compile()
    return nc


def _dispatch(nc, in_maps, time_iters=0):
    """Run the compiled Bass module on NCORES cores via PJRT (axon path),
    mirroring bass2jax.run_bass_via_pjrt but with optional repeat-timing on
    device-resident inputs. Returns (results_list, per_call_seconds)."""
    import jax
    from jax.sharding import Mesh, PartitionSpec
    from jax.experimental.shard_map import shard_map
    from concourse import bass2jax, mybir as _mybir
    from concourse.bass2jax import _bass_exec_p, install_neuronx_cc_hook

    install_neuronx_cc_hook()
    n_cores = len(in_maps)

    partition_name = (nc.partition_id_tensor.name
                      if nc.partition_id_tensor else None)
    in_names, out_names, out_avals, zero_outs = [], [], [], []
    for alloc in nc.m.functions[0].allocations:
        if not isinstance(alloc, _mybir.MemoryLocationSet):
            continue
        name = alloc.memorylocations[0].name
        if alloc.kind == "ExternalInput":
            if name != partition_name:
                in_names.append(name)
        elif alloc.kind == "ExternalOutput":
            shape = tuple(alloc.tensor_shape)
            dtype = _mybir.dt.np(alloc.dtype)
            out_names.append(name)
            out_avals.append(jax.core.ShapedArray(shape, dtype))
            zero_outs.append(np.zeros(shape, dtype))
    n_params = len(in_names)
    all_names = in_names + out_names
    if partition_name is not None:
        all_names = all_names + [partition_name]

    def _body(*args):
        operands = list(args)
        if partition_name is not None:
            operands.append(bass2jax.partition_id_tensor())
        outs = _bass_exec_p.bind(
            *operands,
            out_avals=tuple(out_avals),
            in_names=tuple(all_names),
            out_names=tuple(out_names),
            lowering_input_output_aliases=(),
            sim_require_finite=True,
            sim_require_nnan=True,
            nc=nc,
        )
        return tuple(outs)

    devices = jax.devices()[:n_cores]
    mesh = Mesh(np.asarray(devices), ("core",))
    n_args = n_params + len(out_names)
    sharded = jax.jit(
        shard_map(_body, mesh=mesh,
                  in_specs=(PartitionSpec("core"),) * n_args,
                  out_specs=(PartitionSpec("core"),) * len(out_names),
                  check_rep=False),
        keep_unused=True,
    )
    concat_in = [
        np.concatenate([np.asarray(m[name]) for m in in_maps], axis=0)
        for name in in_names
    ]
    concat_zero = [
        np.zeros((n_cores * z.shape[0], *z.shape[1:]), z.dtype) for z in zero_outs
    ]
    sharding = jax.sharding.NamedSharding(mesh, PartitionSpec("core"))
    dev_args = [jax.device_put(a, sharding) for a in concat_in + concat_zero]

    out_arrs = jax.block_until_ready(sharded(*dev_args))
    times = []
    for _ in range(time_iters):
        t0 = time.perf_counter()
        jax.block_until_ready(sharded(*dev_args))
        times.append(time.perf_counter() - t0)

    results = [
        {name: np.asarray(out_arrs[i]).reshape(n_cores, *out_avals[i].shape)[c]
         for i, name in enumerate(out_names)}
        for c in range(n_cores)
    ]
    return results, times


def _null_nc():
    """Tiny kernel through the same path — measures per-call dispatch floor."""
    nc = bacc.Bacc("TRN2", target_bir_lowering=False, debug=False,
                   num_devices=NCORES)
    x = nc.dram_tensor("x", (128, 128), F32, kind="ExternalInput").ap()
    out = nc.dram_tensor("out", (128, 128), F32, kind="ExternalOutput").ap()
    with tile.TileContext(nc) as tc, ExitStack() as ctx:
        pool = ctx.enter_context(tc.tile_pool(name="p", bufs=1))
        t = pool.tile([128, 128], F32)
        nc.sync.dma_start(t[:], x[:])
        nc.sync.dma_start(out[:], t[:])
    nc.compile()
    return nc


def _prep(input, weight, P1, P2, bias, n_dve=NDVE):
    """Split taps between DVE (largest |coef|, exact fp32) and PE (f32r)."""
    input = np.ascontiguousarray(np.asarray(input, np.float32))
    taps = _host_taps(weight, P1, P2)
    assert len(taps) >= 2
    order = np.argsort([-np.abs(v).mean() for _, v in taps])
    n_dve = max(1, min(n_dve, len(taps) - 1))
    dve_ix = sorted(order[:n_dve])
    pe_ix = sorted(order[n_dve:])
    dve_taps = [taps[j][0] for j in dve_ix]
    pe_taps = [taps[j][0] for j in pe_ix]
    n_pe = len(pe_taps)

    if n_dve:
        coefs = np.stack([taps[j][1] for j in dve_ix], axis=1).astype(np.float32)
    else:
        coefs = np.zeros((C, 1), np.float32)
    diags = np.zeros((NBLK * max(n_pe, 1), 128, 128), np.float32)
    for b in range(NBLK):
        for k, j in enumerate(pe_ix):
            v = taps[j][1][128 * b:128 * (b + 1)].astype(np.float32)
            np.fill_diagonal(diags[b * n_pe + k], v)
    bias_col = np.asarray(bias, np.float32).reshape(C, 1)
    zeros = np.zeros((128, WP * PAD), np.float32)
    in_maps = [
        {"x": input[i * NPER:(i + 1) * NPER], "coefs": coefs, "diags": diags,
         "biasb": bias_col, "zeros": zeros}
        for i in range(NCORES)
    ]
    return dve_taps, pe_taps, in_maps


def _run(input, weight, P1, P2, bias, time_iters=0, n_dve=NDVE):
    dve_taps, pe_taps, in_maps = _prep(input, weight, P1, P2, bias, n_dve=n_dve)
    nc = _build_hybrid(dve_taps, pe_taps)
    results, times = _dispatch(nc, in_maps, time_iters=time_iters)
    full = np.concatenate([r["out"] for r in results], axis=0)
    return full, times


def kernel(input, weight, P1, P2, bias):
    full, _ = _run(input, weight, P1, P2, bias)
    return full


# revision 31
# speedup vs baseline: 1.0179x; 1.0038x over previous
"""Dcls2d depthwise conv (learnable-spacing dilated conv) for Trainium2.

Math: P1/P2 are (1,3,3) -> tap positions shared across all 384 channels.
The 21x21 constructed kernel is a bilinear scatter of the 3x3 weight grid,
so the conv is a sum of <=36 integer-shifted copies of the input, each
scaled by a per-channel coefficient:

    out[n,c,y,x] = bias[c] + sum_j coef[c,j] * in[n,c,y+dy_j,x+dx_j]

Sharding: data-parallel over batch, 32 imgs -> 4 per core on 8 cores.
Shift positions are computed on host from P1/P2 (tiny) and baked into the
compiled kernel's access patterns; per-channel coefficients are a runtime
input tensor.
"""

import os
import time
from contextlib import ExitStack

import numpy as np

import concourse.bass as bass
import concourse.tile as tile
from concourse import bacc, mybir
from concourse.bass_utils import run_bass_kernel_spmd

F32 = mybir.dt.float32
F32R = mybir.dt.float32r
ALU = mybir.AluOpType

N, C, H, W = 32, 384, 56, 56
NCORES = 8
NPER = N // NCORES  # 4 images per core
K0 = K1 = 3
D0 = D1 = 7
L0 = L1 = 21  # constructed kernel size
PAD = 10
NBLK = C // 128  # 3 channel blocks
HP, WP = H + 2 * PAD, W + 2 * PAD  # 76x76 padded tile
RPC = 8                      # output rows per PSUM chunk
NCHUNK = H // RPC            # 7 chunks of 8*56=448 columns
CHW = RPC * W                # 448

# how many taps run as exact-fp32 DVE FMAs (the rest go to the
# TensorEngine as float32r diagonal matmuls); tuned on the cost model.
NDVE = 11


def _host_taps(weight, P1, P2):
    """Bilinear scatter on host -> list of ((dy, dx), coef[384]) taps."""
    w = np.asarray(weight, np.float64).reshape(C, K0 * K1)  # Cg == 1
    p1 = np.clip(np.asarray(P1, np.float64).reshape(-1) + L0 // 2, 0.0, L0 - 1.0)
    p2 = np.clip(np.asarray(P2, np.float64).reshape(-1) + L1 // 2, 0.0, L1 - 1.0)
    f1, f2 = np.floor(p1), np.floor(p2)
    r1, r2 = p1 - f1, p2 - f2
    i1, i2 = f1.astype(int), f2.astype(int)
    i1p = np.minimum(i1 + 1, L0 - 1)
    i2p = np.minimum(i2 + 1, L1 - 1)

    acc = {}  # (a, b) -> coef vector (float64)
    for t in range(K0 * K1):
        for a, b, cf in (
            (i1[t], i2[t], (1 - r1[t]) * (1 - r2[t])),
            (i1p[t], i2[t], r1[t] * (1 - r2[t])),
            (i1[t], i2p[t], (1 - r1[t]) * r2[t]),
            (i1p[t], i2p[t], r1[t] * r2[t]),
        ):
            key = (int(a), int(b))
            v = acc.setdefault(key, np.zeros(C, np.float64))
            v += w[:, t] * cf

    taps = [((a - PAD, b - PAD), v) for (a, b), v in sorted(acc.items())]
    return taps


def _build_hybrid(dve_taps, pe_taps, reps=1):
    """Hybrid TensorE+VectorE per-core program.

    dve_taps: list of (dy, dx) done as exact-fp32 scalar_tensor_tensor on DVE
    pe_taps:  list of (dy, dx) done as float32r diagonal matmuls on TensorE,
              accumulated in PSUM (7 banks = 7 row-chunks per image-block)
    Inputs (x/diags pre-rounded to f32r on host, shipped as raw fp32 bits):
    x (NPER,C,H,W); coefs (C, n_dve); diags (NBLK*n_pe,128,128); biasb (C,1);
    zeros (128, 760).
    """
    n_dve, n_pe = len(dve_taps), len(pe_taps)
    nc = bacc.Bacc("TRN2", target_bir_lowering=False, debug=False,
                   num_devices=NCORES)
    x = nc.dram_tensor("x", (NPER, C, HP, WP), F32R, kind="ExternalInput").ap()
    coefs = nc.dram_tensor("coefs", (C, max(n_dve, 1)), F32,
                           kind="ExternalInput").ap()
    diags = nc.dram_tensor("diags", (NBLK, n_pe, 128, 128), F32R,
                           kind="ExternalInput").ap()
    biasb = nc.dram_tensor("biasb", (C, 1), F32, kind="ExternalInput").ap()
    out = nc.dram_tensor("out", (NPER, C, H, W), F32, kind="ExternalOutput").ap()

    with tile.TileContext(nc) as tc, ExitStack() as ctx:
        cpool = ctx.enter_context(tc.tile_pool(name="const", bufs=NBLK))
        dpool = ctx.enter_context(tc.tile_pool(name="diag", bufs=NBLK * n_pe))
        ppool = ctx.enter_context(tc.tile_pool(name="pad", bufs=3))
        apool = ctx.enter_context(tc.tile_pool(name="acc", bufs=3))
        pspool = ctx.enter_context(tc.tile_pool(name="psum", bufs=8,
                                                space="PSUM"))

        ct, bt = [], []
        for b in range(NBLK):
            c_t = cpool.tile([128, max(n_dve, 1)], F32, tag="coef")
            nc.sync.dma_start(c_t[:], coefs[128 * b:128 * (b + 1), :])
            ct.append(c_t)
            b_t = cpool.tile([128, 1], F32, tag="bias")
            nc.sync.dma_start(b_t[:], biasb[128 * b:128 * (b + 1), :])
            bt.append(b_t)

        dg = {}
        for b in range(NBLK):
            for k in range(n_pe):
                d_t = dpool.tile([128, 128], F32R, tag="diag",
                                 name=f"diag{b}_{k}")
                nc.sync.dma_start(d_t[:], diags[b, k])
                dg[(b, k)] = d_t

        rep_ctx = tc.For_i(0, reps, 1) if reps > 1 else None
        if rep_ctx is not None:
            ctx.enter_context(rep_ctx)
        for i in range(NPER):
            for b in range(NBLK):
                # padded float32r image block, pre-padded+rounded on host
                xp = ppool.tile([128, HP * WP], F32R, tag="xpad")
                xp3 = xp[:].rearrange("c (h w) -> c h w", w=WP)
                nc.sync.dma_start(
                    xp[:],
                    x[i, 128 * b:128 * (b + 1)].rearrange("c h w -> c (h w)"))

                xpf = xp[:].bitcast(F32).rearrange("c (h w) -> c h w", w=WP)

                # --- TensorE: per-tap diagonal matmuls, tap-outer so each
                # diag loads once per image-block; 7 PSUM banks accumulate ---
                pst = [pspool.tile([128, CHW], F32, tag="ps", name=f"ps{cix}")
                       for cix in range(NCHUNK)]
                for k, (dy, dx) in enumerate(pe_taps):
                    d_t = dg[(b, k)]
                    for cix in range(NCHUNK):
                        rhs = xp3[:, RPC * cix + PAD + dy:RPC * (cix + 1) + PAD + dy,
                                  PAD + dx:PAD + dx + W]
                        nc.tensor.matmul(pst[cix][:], d_t[:], rhs,
                                         start=(k == 0), stop=(k == n_pe - 1))

                # --- VectorE: exact fp32 taps into SBUF accumulator ---
                acc = apool.tile([128, H * W], F32)
                a3 = acc[:].rearrange("c (h w) -> c h w", w=W)
                for t, (dy, dx) in enumerate(dve_taps):
                    if t == 0:
                        # full rect: acc = coef * x_shift (borders read zeros)
                        nc.vector.tensor_scalar(
                            a3[:, :, :],
                            xpf[:, PAD + dy:PAD + dy + H, PAD + dx:PAD + dx + W],
                            ct[b][:, 0:1], None, ALU.mult)
                        continue
                    y0, y1 = max(0, -dy), min(H, H - dy)
                    x0, x1 = max(0, -dx), min(W, W - dx)
                    av = a3[:, y0:y1, x0:x1]
                    xv = xpf[:, PAD + y0 + dy:PAD + y1 + dy,
                             PAD + x0 + dx:PAD + x1 + dx]
                    nc.vector.scalar_tensor_tensor(
                        av, xv, ct[b][:, t:t + 1], av, ALU.mult, ALU.add)

                # --- merge PSUM + acc + bias on DVE ---
                for cix in range(NCHUNK):
                    ac = acc[:, CHW * cix:CHW * (cix + 1)]
                    nc.vector.scalar_tensor_tensor(
                        ac, pst[cix][:], bt[b][:, 0:1], ac, ALU.add, ALU.add)

                nc.sync.dma_start(
                    out[i, 128 * b:128 * (b + 1)].rearrange("c h w -> c (h w)"),
                    acc[:])

    nc.compile()
    return nc


def _build(taps, bias_np=None):
    """Build + compile the per-core Bass program. taps: list of (dy, dx)."""
    ntaps = len(taps)
    nc = bacc.Bacc("TRN2", target_bir_lowering=False, debug=False,
                   num_devices=NCORES)
    x = nc.dram_tensor("x", (NPER, C, H, W), F32, kind="ExternalInput").ap()
    coefs = nc.dram_tensor("coefs", (C, ntaps), F32, kind="ExternalInput").ap()
    biasb = nc.dram_tensor("biasb", (C, 1), F32, kind="ExternalInput").ap()
    out = nc.dram_tensor("out", (NPER, C, H, W), F32, kind="ExternalOutput").ap()

    with tile.TileContext(nc) as tc, ExitStack() as ctx:
        cpool = ctx.enter_context(tc.tile_pool(name="const", bufs=NBLK))
        xpool = ctx.enter_context(tc.tile_pool(name="xin", bufs=3))
        apool = ctx.enter_context(tc.tile_pool(name="acc", bufs=3))

        ct, bt = [], []
        for b in range(NBLK):
            c_t = cpool.tile([128, ntaps], F32, tag="coef")
            nc.sync.dma_start(c_t[:], coefs[128 * b:128 * (b + 1), :])
            ct.append(c_t)
            b_t = cpool.tile([128, 1], F32, tag="bias")
            nc.sync.dma_start(b_t[:], biasb[128 * b:128 * (b + 1), :])
            bt.append(b_t)

        for i in range(NPER):
            for b in range(NBLK):
                xt = xpool.tile([128, H * W], F32)
                nc.sync.dma_start(
                    xt[:],
                    x[i, 128 * b:128 * (b + 1)].rearrange("c h w -> c (h w)"))
                acc = apool.tile([128, H * W], F32)
                # acc = x*0 + bias  (broadcast per-partition bias)
                nc.vector.tensor_scalar(acc[:], xt[:], 0.0, bt[b][:, 0:1],
                                        ALU.mult, ALU.add)
                x3 = xt[:].rearrange("c (h w) -> c h w", w=W)
                a3 = acc[:].rearrange("c (h w) -> c h w", w=W)
                for t, (dy, dx) in enumerate(taps):
                    y0, y1 = max(0, -dy), min(H, H - dy)
                    x0, x1 = max(0, -dx), min(W, W - dx)
                    av = a3[:, y0:y1, x0:x1]
                    xv = x3[:, y0 + dy:y1 + dy, x0 + dx:x1 + dx]
                    # acc += coef[c,t] * x_shifted   (one fused DVE op)
                    nc.vector.scalar_tensor_tensor(
                        av, xv, ct[b][:, t:t + 1], av, ALU.mult, ALU.add)
                nc.sync.dma_start(
                    out[i, 128 * b:128 * (b + 1)].rearrange("c h w -> c (h w)"),
                    acc[:])

    nc.compile()
    return nc


def _dispatch(nc, in_maps, time_iters=0):
    """Run the compiled Bass module on NCORES cores via PJRT (axon path),
    mirroring bass2jax.run_bass_via_pjrt but with optional repeat-timing on
    device-resident inputs. Returns (results_list, per_call_seconds)."""
    import jax
    from jax.sharding import Mesh, PartitionSpec
    from jax.experimental.shard_map import shard_map
    from concourse import bass2jax, mybir as _mybir
    from concourse.bass2jax import _bass_exec_p, install_neuronx_cc_hook

    install_neuronx_cc_hook()
    n_cores = len(in_maps)

    partition_name = (nc.partition_id_tensor.name
                      if nc.partition_id_tensor else None)
    in_names, out_names, out_avals, zero_outs = [], [], [], []
    for alloc in nc.m.functions[0].allocations:
        if not isinstance(alloc, _mybir.MemoryLocationSet):
            continue
        name = alloc.memorylocations[0].name
        if alloc.kind == "ExternalInput":
            if name != partition_name:
                in_names.append(name)
        elif alloc.kind == "ExternalOutput":
            shape = tuple(alloc.tensor_shape)
            dtype = _mybir.dt.np(alloc.dtype)
            out_names.append(name)
            out_avals.append(jax.core.ShapedArray(shape, dtype))
            zero_outs.append(np.zeros(shape, dtype))
    n_params = len(in_names)
    all_names = in_names + out_names
    if partition_name is not None:
        all_names = all_names + [partition_name]

    def _body(*args):
        operands = list(args)
        if partition_name is not None:
            operands.append(bass2jax.partition_id_tensor())
        outs = _bass_exec_p.bind(
            *operands,
            out_avals=tuple(out_avals),
            in_names=tuple(all_names),
            out_names=tuple(out_names),
            lowering_input_output_aliases=(),
            sim_require_finite=True,
            sim_require_nnan=True,
            nc=nc,
        )
        return tuple(outs)

    devices = jax.devices()[:n_cores]
    mesh = Mesh(np.asarray(devices), ("core",))
    n_args = n_params + len(out_names)
    sharded = jax.jit(
        shard_map(_body, mesh=mesh,
                  in_specs=(PartitionSpec("core"),) * n_args,
                  out_specs=(PartitionSpec("core"),) * len(out_names),
                  check_rep=False),
        keep_unused=True,
    )
    concat_in = [
        np.concatenate([np.asarray(m[name]) for m in in_maps], axis=0)
        for name in in_names
    ]
    concat_zero = [
        np.zeros((n_cores * z.shape[0], *z.shape[1:]), z.dtype) for z in zero_outs
    ]
    sharding = jax.sharding.NamedSharding(mesh, PartitionSpec("core"))
    dev_args = [jax.device_put(a, sharding) for a in concat_in + concat_zero]

    out_arrs = jax.block_until_ready(sharded(*dev_args))
    times = []
    for _ in range(time_iters):
        t0 = time.perf_counter()
        jax.block_until_ready(sharded(*dev_args))
        times.append(time.perf_counter() - t0)

    results = [
        {name: np.asarray(out_arrs[i]).reshape(n_cores, *out_avals[i].shape)[c]
         for i, name in enumerate(out_names)}
        for c in range(n_cores)
    ]
    return results, times


def _null_nc():
    """Tiny kernel through the same path — measures per-call dispatch floor."""
    nc = bacc.Bacc("TRN2", target_bir_lowering=False, debug=False,
                   num_devices=NCORES)
    x = nc.dram_tensor("x", (128, 128), F32, kind="ExternalInput").ap()
    out = nc.dram_tensor("out", (128, 128), F32, kind="ExternalOutput").ap()
    with tile.TileContext(nc) as tc, ExitStack() as ctx:
        pool = ctx.enter_context(tc.tile_pool(name="p", bufs=1))
        t = pool.tile([128, 128], F32)
        nc.sync.dma_start(t[:], x[:])
        nc.sync.dma_start(out[:], t[:])
    nc.compile()
    return nc


def _round_f32r(a):
    """RNE to 11 mantissa bits — the float32r storage format (HW-verified)."""
    b = np.ascontiguousarray(np.asarray(a, np.float32)).view(np.uint32)
    sh = 12
    lsb = (b >> sh) & 1
    r = ((b + np.uint32((1 << (sh - 1)) - 1) + lsb) >> sh) << sh
    return r.view(np.float32)


def _prep(input, weight, P1, P2, bias, n_dve=NDVE):
    """Split taps between DVE (largest |coef|, exact fp32) and PE (f32r)."""
    input = _round_f32r(input)
    input = np.pad(input.reshape(N, C, H, W),
                   ((0, 0), (0, 0), (PAD, PAD), (PAD, PAD)))
    taps = _host_taps(weight, P1, P2)
    assert len(taps) >= 2
    order = np.argsort([-np.abs(v).mean() for _, v in taps])
    n_dve = max(1, min(n_dve, len(taps) - 1))
    dve_ix = sorted(order[:n_dve])
    pe_ix = sorted(order[n_dve:])
    dve_taps = [taps[j][0] for j in dve_ix]
    pe_taps = [taps[j][0] for j in pe_ix]
    n_pe = len(pe_taps)

    if n_dve:
        coefs = np.stack([taps[j][1] for j in dve_ix], axis=1).astype(np.float32)
    else:
        coefs = np.zeros((C, 1), np.float32)
    diags = np.zeros((NBLK, max(n_pe, 1), 128, 128), np.float32)
    for b in range(NBLK):
        for k, j in enumerate(pe_ix):
            v = _round_f32r(taps[j][1].astype(np.float32)[128 * b:128 * (b + 1)])
            np.fill_diagonal(diags[b, k], v)
    bias_col = np.asarray(bias, np.float32).reshape(C, 1)
    in_maps = [
        {"x": input[i * NPER:(i + 1) * NPER], "coefs": coefs, "diags": diags,
         "biasb": bias_col}
        for i in range(NCORES)
    ]
    return dve_taps, pe_taps, in_maps


def _run(input, weight, P1, P2, bias, time_iters=0, n_dve=NDVE):
    dve_taps, pe_taps, in_maps = _prep(input, weight, P1, P2, bias, n_dve=n_dve)
    nc = _build_hybrid(dve_taps, pe_taps)
    results, times = _dispatch(nc, in_maps, time_iters=time_iters)
    full = np.concatenate([r["out"] for r in results], axis=0)
    return full, times


def kernel(input, weight, P1, P2, bias):
    full, _ = _run(input, weight, P1, P2, bias)
    return full
